# revision 19
# baseline (speedup 1.0000x reference)
"""CARAFE-downsampling Trainium2 kernel (8-core SPMD, full I/O contract).

Per core (core = 4n + s; batch n, output-row slab h' in [32s, 32s+32)):

  enc logits fused down+enc (9 taps, C_tap = B_tap @ A on host):
      enc[e, hd, wd] = sum_tap C_tap.T @ xk[:, 2hd+dy, 2wd+dx]
      xk = x rows [64s-1, 64s+64) + mask channel, columns pre-deinterleaved
      (even/odd) on host so matmul rhs reads are step-1.
  kw = softmax_k(enc) computed in [k-partition, (hd, wd)-free] layout:
      exp on ACT, sum-over-k via ones-matmul on PE, reciprocal + normalize
      on DVE.  kw -> DRAM scratch -> partition-broadcast DMA back as
      kwb[(q,co), hh, w'] tiles (64-way partition replication).
  G[co, r, u] = sum_{c,t} out_w[co, 4c+t] x[c, 64t+16s-2+r, u-2]  (PE)
      evicted bf16, then DMA'd into five kj-shifted flat tiles
      g2q_kj[(q,co), r, w'] = G[co, r, 128q+w'+kj-2].
  products: per half H (hh in [8H, 8H+8)), 25 taps k=(ki,kj):
      stg = g2q_kj[:, 8H+ki : +8, :] * kwb  (flat [128,1024] bf16;
      DVE 2x-mode / gpsimd split), accumulated with identity matmuls
      into PSUM on PE; out_b added during ACT eviction (bias vector).
"""
import os

import numpy as np
import ml_dtypes

import concourse.bass as bass
import concourse.tile as tile
from concourse import bacc, mybir, masks
from concourse.bass_utils import run_bass_kernel_spmd

F32 = mybir.dt.float32
BF16 = mybir.dt.bfloat16

N_CORES = 8


# ----------------------------------------------------------------------------
# device program
# ----------------------------------------------------------------------------
def build_nc():
    nc = bacc.Bacc(None, target_bir_lowering=False)

    xk_d = nc.dram_tensor("xk", [2, 65, 65, 129], BF16, kind="ExternalInput")
    xb_d = nc.dram_tensor("xb", [2, 128, 20, 264], BF16, kind="ExternalInput")
    ct_d = nc.dram_tensor("ctap", [65, 9, 25], BF16, kind="ExternalInput")
    w4_d = nc.dram_tensor("w4", [2, 128, 64], BF16, kind="ExternalInput")
    ob_d = nc.dram_tensor("obv", [128, 1], F32, kind="ExternalInput")
    kwd_d = nc.dram_tensor("kwd", [2, 2, 25, 8, 128], BF16, kind="Internal")
    # out[H, (q,co), hh, w']; h' = 16H + 2hh + q
    out_d = nc.dram_tensor("out", [2, 128, 8, 128], BF16, kind="ExternalOutput")

    ctx = nc.allow_low_precision(reason="bf16 pipeline; validated ~1% rel err")
    ctx.__enter__()
    with tile.TileContext(nc) as tc:
        with (
            tc.tile_pool(name="consts", bufs=1) as consts,
            tc.tile_pool(name="xkp", bufs=8) as xkp,
            tc.tile_pool(name="xbp", bufs=1) as xbp,
            tc.tile_pool(name="gbfp", bufs=1) as gbfp,
            tc.tile_pool(name="g2qp", bufs=1) as g2qp,
            tc.tile_pool(name="kwp", bufs=1) as kwp,
            tc.tile_pool(name="kwbp", bufs=1) as kwbp,
            tc.tile_pool(name="stgp", bufs=8) as stgp,
            tc.tile_pool(name="resp", bufs=2) as resp,
            tc.tile_pool(name="pse", bufs=3, space="PSUM") as pse,
            tc.tile_pool(name="psg", bufs=2, space="PSUM") as psg,
            tc.tile_pool(name="pss", bufs=1, space="PSUM") as pss,
            tc.tile_pool(name="psacc", bufs=1, space="PSUM") as psacc,
        ):
            # ---- constants ----
            ctap = consts.tile([65, 9, 25], BF16)
            nc.sync.dma_start(ctap[:], ct_d[:])
            w4t = consts.tile([128, 2, 64], BF16)
            nc.scalar.dma_start(w4t[:], w4_d[:].transpose([1, 0, 2]))
            obv = consts.tile([128, 1], F32)
            nc.sync.dma_start(obv[:], ob_d[:])
            identb = consts.tile([128, 128], BF16)
            masks.make_identity(nc, identb[:])
            ones25 = consts.tile([25, 25], BF16)
            nc.gpsimd.memset(ones25[:], 1.0)

            # ---- input streams ----
            xbt = xbp.tile([128, 2, 20, 264], BF16)
            nc.scalar.dma_start(xbt[:], xb_d[:].transpose([1, 0, 2, 3]))
            xkc = [[], []]
            for cc in range(8):
                for par in range(2):
                    t = xkp.tile([65, 10, 129], BF16, tag=f"xk{par}",
                                 name=f"xk{par}_{cc}")
                    nr = 10 if cc < 7 else 9
                    nc.sync.dma_start(t[:, 0:nr, :],
                                       xk_d[par, :, 8 * cc:8 * cc + nr, :])
                    xkc[par].append(t)

            kwe = kwp.tile([25, 32, 128], BF16)
            kwn = kwp.tile([25, 32, 128], BF16)
            rcp = kwp.tile([25, 32, 128], F32)
            gbf = gbfp.tile([64, 20, 264], BF16)
            g2q = [g2qp.tile([128, 20, 128], BF16, name=f"g2q{kj}",
                             tag=f"g2q{kj}") for kj in range(5)]

            def enc_chunk(cc):
                pe = pse.tile([25, 4, 128], F32, name=f"pe{cc}", tag="pe")
                first = True
                for dy in range(3):
                    for dx in range(3):
                        par, off = dx % 2, dx // 2
                        rhs = xkc[par][cc][:, dy:dy + 8:2, off:off + 128]
                        nc.tensor.matmul(
                            pe[:], ctap[:, 3 * dy + dx, :], rhs,
                            start=first, stop=(dy == 2 and dx == 2))
                        first = False
                nc.scalar.activation(kwe[:, 4 * cc:4 * cc + 4, :], pe[:],
                                     mybir.ActivationFunctionType.Exp)

            def sum_chunk(cc):
                ps = pss.tile([25, 4, 128], F32, name=f"ps{cc}", tag="ps")
                nc.tensor.matmul(ps[:], ones25[:],
                                 kwe[:, 4 * cc:4 * cc + 4, :],
                                 start=True, stop=True)
                nc.vector.reciprocal_approx_fast(
                    rcp[:, 4 * cc:4 * cc + 4, :], ps[:])
                nc.vector.tensor_mul(kwn[:, 4 * cc:4 * cc + 4, :],
                                     kwe[:, 4 * cc:4 * cc + 4, :],
                                     rcp[:, 4 * cc:4 * cc + 4, :])

            def g_rows(rs):
                pgs = [psg.tile([64, 264], F32, tag="pg", name=f"pg{r}")
                       for r in rs]
                for ci in range(2):
                    for i, r in enumerate(rs):
                        nc.tensor.matmul(pgs[i][:], w4t[:, ci, :],
                                         xbt[:, ci, r, :],
                                         start=(ci == 0), stop=(ci == 1))
                for i, r in enumerate(rs):
                    nc.scalar.copy(gbf[:, r, :], pgs[i][:])

            # ---- phase 1: enc 0-3 + sums (kw-H0 critical path) ----
            for cc in range(4):
                enc_chunk(cc)
                sum_chunk(cc)

            # ---- kw half export + broadcast-back ----
            kwb = {}

            def kw_half(H):
                # kwn rows hd in [16H, 16H+16) -> kwd[H][q, k, hh, w']
                for q in range(2):
                    nc.sync.dma_start(
                        kwd_d[H, q],
                        kwn[:, 16 * H + q:16 * H + 16:2, :])
                for g in range(5):
                    t = kwbp.tile([128, 5, 8, 128], BF16, name=f"kwb{H}{g}",
                                  tag=f"kwb{g}")
                    for q in range(2):
                        eng = nc.sync if H == 0 else nc.scalar
                        eng.dma_start(
                            t[64 * q:64 * q + 64],
                            kwd_d[H, q, g * 5:g * 5 + 5].unsqueeze(0)
                            .broadcast_to([64, 5, 8, 128]))
                    kwb[(H, g)] = t

            # ---- G rows ----
            for j in range(0, 20, 2):
                g_rows([j, j + 1])

            # ---- g2q shifted tiles (DMA partition remap, gpsimd SWDGE) ----
            for kj in range(5):
                for q in range(2):
                    nc.gpsimd.dma_start(
                        g2q[kj][64 * q:64 * q + 64, :, :],
                        gbf[:, :, 128 * q + kj:128 * q + kj + 128])

            kw_half(0)

            # ---- phase 2: enc 4-7 ----
            for cc in range(4, 8):
                enc_chunk(cc)
                sum_chunk(cc)
            kw_half(1)

            # ---- products ----
            def prod_half(H):
                acc = [psacc.tile([128, 512], F32, name=f"acc{H}{b}",
                                  tag=f"acc{b}") for b in range(2)]
                for k in range(25):
                    ki, kj = divmod(k, 5)
                    stg = stgp.tile([128, 1024], BF16, tag="stg",
                                    name=f"stg{H}_{k}")
                    gsl = (g2q[kj][:, 8 * H + ki:8 * H + ki + 8, :]
                           .rearrange("p h w -> p (h w)"))
                    wsl = (kwb[(H, ki)][:, k - 5 * ki]
                           .rearrange("p h w -> p (h w)"))
                    nc.vector.tensor_mul(stg[:], gsl, wsl)
                    for b in range(2):
                        nc.tensor.matmul(acc[b][:], identb[:],
                                         stg[:, 512 * b:512 * b + 512],
                                         start=(k == 0), stop=(k == 24))
                res = resp.tile([128, 8, 128], BF16, tag="res",
                                name=f"res{H}")
                for b in range(2):
                    nc.scalar.activation(
                        res[:, 4 * b:4 * b + 4, :], acc[b][:],
                        mybir.ActivationFunctionType.Identity,
                        bias=obv[:, 0].unsqueeze(-1))
                eng = nc.sync if H == 0 else nc.scalar
                eng.dma_start(out_d[H], res[:])

            prod_half(0)
            prod_half(1)

    nc.compile()
    ctx.__exit__(None, None, None)
    return nc


# ----------------------------------------------------------------------------
# host side
# ----------------------------------------------------------------------------
def _prep_weights(down_w, down_b, enc_w, enc_b, out_w, out_b):
    A = np.zeros((65, 65), np.float32)
    A[0:64, 0:64] = down_w[:, :, 0, 0]
    A[0:64, 64] = down_b
    A[64, 64] = 1.0
    ctap = np.zeros((65, 9, 25), np.float32)
    for dy in range(3):
        for dx in range(3):
            B = np.zeros((25, 65), np.float32)
            B[:, 0:64] = enc_w[:, :, dy, dx]
            if dy == 1 and dx == 1:
                B[:, 64] = enc_b
            ctap[:, 3 * dy + dx, :] = (B @ A).T
    w4 = out_w[:, :, 0, 0].T.reshape(2, 128, 64).astype(ml_dtypes.bfloat16)
    obv = np.tile(out_b, 2).reshape(128, 1).astype(np.float32)
    return ctap.astype(ml_dtypes.bfloat16), w4, obv


def _slice_core(x, n, s):
    xk = np.zeros((65, 65, 258), np.float32)
    h0 = 64 * s - 1
    lo, hi = max(0, -h0), min(65, 256 - h0)
    xk[0:64, lo:hi, 1:257] = x[n, :, h0 + lo:h0 + hi, :]
    xk[64, lo:hi, 1:257] = 1.0
    xkp = np.zeros((2, 65, 65, 129), np.float32)
    xkp[0] = xk[:, :, 0::2]
    xkp[1] = xk[:, :, 1::2]
    xb = np.zeros((2, 128, 20, 264), np.float32)
    xbv = xb.reshape(256, 20, 264)
    for t in range(4):
        g0 = 64 * t + 16 * s - 2
        lo, hi = max(0, -g0), min(20, 256 - g0)
        xbv[np.arange(64) * 4 + t, lo:hi, 2:258] = x[n, :, g0 + lo:g0 + hi, :]
    return xkp.astype(ml_dtypes.bfloat16), xb.astype(ml_dtypes.bfloat16)


_NC_CACHE = None
LAST_EXEC_NS = None


def kernel(x, down_w, down_b, enc_w, enc_b, out_w, out_b):
    global _NC_CACHE, LAST_EXEC_NS
    x = np.asarray(x, np.float32)
    ctap, w4, obv = _prep_weights(
        np.asarray(down_w, np.float32), np.asarray(down_b, np.float32),
        np.asarray(enc_w, np.float32), np.asarray(enc_b, np.float32),
        np.asarray(out_w, np.float32), np.asarray(out_b, np.float32))
    in_maps = []
    for core in range(N_CORES):
        n, s = core // 4, core % 4
        xkp, xb = _slice_core(x, n, s)
        in_maps.append({"xk": xkp, "xb": xb, "ctap": ctap, "w4": w4,
                        "obv": obv})
    if _NC_CACHE is None:
        _NC_CACHE = build_nc()
    kw = {}
    if os.environ.get("CARAFE_TRACE"):
        kw = dict(trace=True, tmpdir=os.environ.get("CARAFE_TRACE_DIR"))
    res = run_bass_kernel_spmd(_NC_CACHE, in_maps, list(range(N_CORES)), **kw)
    if res.exec_time_ns is not None:
        LAST_EXEC_NS = res.exec_time_ns
    out = np.zeros((2, 64, 128, 128), np.float32)
    for core in range(N_CORES):
        n, s = core // 4, core % 4
        o = res.results[core]["out"].astype(np.float32)  # (H, (q,co), hh, w')
        o = o.reshape(2, 2, 64, 8, 128)                  # (H, q, co, hh, w')
        # h' = 16H + 2hh + q
        o = o.transpose(2, 0, 3, 1, 4).reshape(64, 32, 128)
        out[n, :, 32 * s:32 * s + 32, :] = o
    return out


# revision 21
# speedup vs baseline: 1.0450x; 1.0450x over previous
"""CARAFE-downsampling Trainium2 kernel (8-core SPMD, full I/O contract).

Per core (core = 4n + s; batch n, output-row slab h' in [32s, 32s+32)):

  enc logits fused down+enc (9 taps, C_tap = B_tap @ A on host):
      enc[e, hd, wd] = sum_tap C_tap.T @ xk[:, 2hd+dy, 2wd+dx]
      xk = x rows [64s-1, 64s+64) + mask channel, columns pre-deinterleaved
      (even/odd) on host so matmul rhs reads are step-1.
  kw = softmax_k(enc) computed in [k-partition, (hd, wd)-free] layout:
      exp on ACT, sum-over-k via ones-matmul on PE, reciprocal + normalize
      on DVE.  kw -> DRAM scratch -> partition-broadcast DMA back as
      kwb[(q,co), hh, w'] tiles (64-way partition replication).
  G[co, r, u] = sum_{c,t} out_w[co, 4c+t] x[c, 64t+16s-2+r, u-2]  (PE)
      evicted bf16, then DMA'd into five kj-shifted flat tiles
      g2q_kj[(q,co), r, w'] = G[co, r, 128q+w'+kj-2].
  products: per half H (hh in [8H, 8H+8)), 25 taps k=(ki,kj):
      stg = g2q_kj[:, 8H+ki : +8, :] * kwb  (flat [128,1024] bf16;
      DVE 2x-mode / gpsimd split), accumulated with identity matmuls
      into PSUM on PE; out_b added during ACT eviction (bias vector).
"""
import os

import numpy as np
import ml_dtypes

import concourse.bass as bass
import concourse.tile as tile
from concourse import bacc, mybir, masks
from concourse.bass_utils import run_bass_kernel_spmd

F32 = mybir.dt.float32
BF16 = mybir.dt.bfloat16

N_CORES = 8


# ----------------------------------------------------------------------------
# device program
# ----------------------------------------------------------------------------
def build_nc():
    nc = bacc.Bacc(None, target_bir_lowering=False)

    xk_d = nc.dram_tensor("xk", [2, 65, 65, 129], BF16, kind="ExternalInput")
    xb_d = nc.dram_tensor("xb", [2, 128, 20, 264], BF16, kind="ExternalInput")
    ct_d = nc.dram_tensor("ctap", [65, 9, 25], BF16, kind="ExternalInput")
    w4_d = nc.dram_tensor("w4", [2, 128, 64], BF16, kind="ExternalInput")
    ob_d = nc.dram_tensor("obv", [128, 1], F32, kind="ExternalInput")
    kwd_d = nc.dram_tensor("kwd", [2, 2, 25, 8, 128], BF16, kind="Internal")
    # out[H, (q,co), hh, w']; h' = 16H + 2hh + q
    out_d = nc.dram_tensor("out", [2, 128, 8, 128], BF16, kind="ExternalOutput")

    ctx = nc.allow_low_precision(reason="bf16 pipeline; validated ~1% rel err")
    ctx.__enter__()
    with tile.TileContext(nc) as tc:
        with (
            tc.tile_pool(name="consts", bufs=1) as consts,
            tc.tile_pool(name="xkp", bufs=4) as xkp,
            tc.tile_pool(name="xbp", bufs=1) as xbp,
            tc.tile_pool(name="gbfp", bufs=1) as gbfp,
            tc.tile_pool(name="g2qp", bufs=1) as g2qp,
            tc.tile_pool(name="kwp", bufs=1) as kwp,
            tc.tile_pool(name="kwbp", bufs=1) as kwbp,
            tc.tile_pool(name="stgp", bufs=8) as stgp,
            tc.tile_pool(name="resp", bufs=2) as resp,
            tc.tile_pool(name="pse", bufs=2, space="PSUM") as pse,
            tc.tile_pool(name="psg", bufs=2, space="PSUM") as psg,
            tc.tile_pool(name="pss", bufs=2, space="PSUM") as pss,
            tc.tile_pool(name="psacc", bufs=1, space="PSUM") as psacc,
        ):
            # ---- constants ----
            ctap = consts.tile([65, 9, 25], BF16)
            nc.sync.dma_start(ctap[:], ct_d[:])
            w4t = consts.tile([128, 2, 64], BF16)
            nc.scalar.dma_start(w4t[:], w4_d[:].transpose([1, 0, 2]))
            obv = consts.tile([128, 1], F32)
            nc.sync.dma_start(obv[:], ob_d[:])
            identb = consts.tile([128, 128], BF16)
            masks.make_identity(nc, identb[:])
            ones25 = consts.tile([25, 25], BF16)
            nc.gpsimd.memset(ones25[:], 1.0)

            # ---- input streams ----
            xbt = xbp.tile([128, 2, 20, 264], BF16)
            nc.scalar.dma_start(xbt[:], xb_d[:].transpose([1, 0, 2, 3]))
            xkc = [[], []]
            for cc in range(8):
                for par in range(2):
                    t = xkp.tile([65, 10, 129], BF16, tag=f"xk{par}",
                                 name=f"xk{par}_{cc}")
                    nr = 10 if cc < 7 else 9
                    nc.sync.dma_start(t[:, 0:nr, :],
                                       xk_d[par, :, 8 * cc:8 * cc + nr, :])
                    xkc[par].append(t)

            kwe = kwp.tile([25, 32, 128], BF16)
            kwn = kwp.tile([25, 32, 128], BF16)
            rcp = kwp.tile([25, 32, 128], F32)
            gbf = gbfp.tile([64, 20, 264], BF16)
            g2q = [g2qp.tile([128, 20, 128], BF16, name=f"g2q{kj}",
                             tag=f"g2q{kj}") for kj in range(5)]

            def enc_chunk(cc):
                pe = pse.tile([25, 4, 128], F32, name=f"pe{cc}", tag="pe")
                first = True
                for dy in range(3):
                    for dx in range(3):
                        par, off = dx % 2, dx // 2
                        rhs = xkc[par][cc][:, dy:dy + 8:2, off:off + 128]
                        nc.tensor.matmul(
                            pe[:], ctap[:, 3 * dy + dx, :], rhs,
                            start=first, stop=(dy == 2 and dx == 2))
                        first = False
                nc.scalar.activation(kwe[:, 4 * cc:4 * cc + 4, :], pe[:],
                                     mybir.ActivationFunctionType.Exp)

            def sum_chunk(cc):
                ps = pss.tile([25, 4, 128], F32, name=f"ps{cc}", tag="ps")
                nc.tensor.matmul(ps[:], ones25[:],
                                 kwe[:, 4 * cc:4 * cc + 4, :],
                                 start=True, stop=True)
                nc.vector.reciprocal_approx_fast(
                    rcp[:, 4 * cc:4 * cc + 4, :], ps[:])
                nc.vector.tensor_mul(kwn[:, 4 * cc:4 * cc + 4, :],
                                     kwe[:, 4 * cc:4 * cc + 4, :],
                                     rcp[:, 4 * cc:4 * cc + 4, :])

            def g_rows(rs):
                pgs = [psg.tile([64, 264], F32, tag="pg", name=f"pg{r}")
                       for r in rs]
                for ci in range(2):
                    for i, r in enumerate(rs):
                        nc.tensor.matmul(pgs[i][:], w4t[:, ci, :],
                                         xbt[:, ci, r, :],
                                         start=(ci == 0), stop=(ci == 1))
                for i, r in enumerate(rs):
                    nc.scalar.copy(gbf[:, r, :], pgs[i][:])

            # ---- phase 1: enc 0-3 + sums (kw-H0 critical path) ----
            for cc in range(4):
                enc_chunk(cc)
                sum_chunk(cc)

            # ---- kw half export + broadcast-back ----
            kwb = {}

            def kw_half(H):
                # kwn rows hd in [16H, 16H+16) -> kwd[H][q, k, hh, w']
                for q in range(2):
                    nc.sync.dma_start(
                        kwd_d[H, q],
                        kwn[:, 16 * H + q:16 * H + 16:2, :])
                for g in range(5):
                    t = kwbp.tile([128, 5, 8, 128], BF16, name=f"kwb{H}{g}",
                                  tag=f"kwb{g}")
                    for q in range(2):
                        eng = nc.sync if H == 0 else nc.scalar
                        for k0, k1 in ((0, 2), (2, 5)):
                            eng.dma_start(
                                t[64 * q:64 * q + 64, k0:k1],
                                kwd_d[H, q, g * 5 + k0:g * 5 + k1]
                                .unsqueeze(0)
                                .broadcast_to([64, k1 - k0, 8, 128]))
                    kwb[(H, g)] = t

            # ---- G rows ----
            for j in range(0, 20, 2):
                g_rows([j, j + 1])

            # ---- g2q shifted tiles (DMA partition remap, gpsimd SWDGE) ----
            for kj in range(5):
                for q in range(2):
                    nc.gpsimd.dma_start(
                        g2q[kj][64 * q:64 * q + 64, :, :],
                        gbf[:, :, 128 * q + kj:128 * q + kj + 128])

            kw_half(0)

            # ---- phase 2: enc 4-7 ----
            for cc in range(4, 8):
                enc_chunk(cc)
                sum_chunk(cc)
            kw_half(1)

            # ---- products ----
            def prod_half(H):
                acc = [psacc.tile([128, 512], F32, name=f"acc{H}{b}",
                                  tag=f"acc{b}") for b in range(2)]
                for k in range(25):
                    ki, kj = divmod(k, 5)
                    stg = stgp.tile([128, 1024], BF16, tag="stg",
                                    name=f"stg{H}_{k}")
                    gsl = (g2q[kj][:, 8 * H + ki:8 * H + ki + 8, :]
                           .rearrange("p h w -> p (h w)"))
                    wsl = (kwb[(H, ki)][:, k - 5 * ki]
                           .rearrange("p h w -> p (h w)"))
                    nc.vector.tensor_mul(stg[:], gsl, wsl)
                    for b in range(2):
                        nc.tensor.matmul(acc[b][:], identb[:],
                                         stg[:, 512 * b:512 * b + 512],
                                         start=(k == 0), stop=(k == 24))
                res = resp.tile([128, 8, 128], BF16, tag="res",
                                name=f"res{H}")
                for b in range(2):
                    nc.scalar.activation(
                        res[:, 4 * b:4 * b + 4, :], acc[b][:],
                        mybir.ActivationFunctionType.Identity,
                        bias=obv[:, 0].unsqueeze(-1))
                eng = nc.sync if H == 0 else nc.scalar
                eng.dma_start(out_d[H], res[:])

            prod_half(0)
            prod_half(1)

    nc.compile()
    ctx.__exit__(None, None, None)
    return nc


# ----------------------------------------------------------------------------
# host side
# ----------------------------------------------------------------------------
def _prep_weights(down_w, down_b, enc_w, enc_b, out_w, out_b):
    A = np.zeros((65, 65), np.float32)
    A[0:64, 0:64] = down_w[:, :, 0, 0]
    A[0:64, 64] = down_b
    A[64, 64] = 1.0
    ctap = np.zeros((65, 9, 25), np.float32)
    for dy in range(3):
        for dx in range(3):
            B = np.zeros((25, 65), np.float32)
            B[:, 0:64] = enc_w[:, :, dy, dx]
            if dy == 1 and dx == 1:
                B[:, 64] = enc_b
            ctap[:, 3 * dy + dx, :] = (B @ A).T
    w4 = out_w[:, :, 0, 0].T.reshape(2, 128, 64).astype(ml_dtypes.bfloat16)
    obv = np.tile(out_b, 2).reshape(128, 1).astype(np.float32)
    return ctap.astype(ml_dtypes.bfloat16), w4, obv


def _slice_core(x, n, s):
    xk = np.zeros((65, 65, 258), np.float32)
    h0 = 64 * s - 1
    lo, hi = max(0, -h0), min(65, 256 - h0)
    xk[0:64, lo:hi, 1:257] = x[n, :, h0 + lo:h0 + hi, :]
    xk[64, lo:hi, 1:257] = 1.0
    xkp = np.zeros((2, 65, 65, 129), np.float32)
    xkp[0] = xk[:, :, 0::2]
    xkp[1] = xk[:, :, 1::2]
    xb = np.zeros((2, 128, 20, 264), np.float32)
    xbv = xb.reshape(256, 20, 264)
    for t in range(4):
        g0 = 64 * t + 16 * s - 2
        lo, hi = max(0, -g0), min(20, 256 - g0)
        xbv[np.arange(64) * 4 + t, lo:hi, 2:258] = x[n, :, g0 + lo:g0 + hi, :]
    return xkp.astype(ml_dtypes.bfloat16), xb.astype(ml_dtypes.bfloat16)


_NC_CACHE = None
LAST_EXEC_NS = None


def kernel(x, down_w, down_b, enc_w, enc_b, out_w, out_b):
    global _NC_CACHE, LAST_EXEC_NS
    x = np.asarray(x, np.float32)
    ctap, w4, obv = _prep_weights(
        np.asarray(down_w, np.float32), np.asarray(down_b, np.float32),
        np.asarray(enc_w, np.float32), np.asarray(enc_b, np.float32),
        np.asarray(out_w, np.float32), np.asarray(out_b, np.float32))
    in_maps = []
    for core in range(N_CORES):
        n, s = core // 4, core % 4
        xkp, xb = _slice_core(x, n, s)
        in_maps.append({"xk": xkp, "xb": xb, "ctap": ctap, "w4": w4,
                        "obv": obv})
    if _NC_CACHE is None:
        _NC_CACHE = build_nc()
    kw = {}
    if os.environ.get("CARAFE_TRACE"):
        kw = dict(trace=True, tmpdir=os.environ.get("CARAFE_TRACE_DIR"))
    res = run_bass_kernel_spmd(_NC_CACHE, in_maps, list(range(N_CORES)), **kw)
    if res.exec_time_ns is not None:
        LAST_EXEC_NS = res.exec_time_ns
    out = np.zeros((2, 64, 128, 128), np.float32)
    for core in range(N_CORES):
        n, s = core // 4, core % 4
        o = res.results[core]["out"].astype(np.float32)  # (H, (q,co), hh, w')
        o = o.reshape(2, 2, 64, 8, 128)                  # (H, q, co, hh, w')
        # h' = 16H + 2hh + q
        o = o.transpose(2, 0, 3, 1, 4).reshape(64, 32, 128)
        out[n, :, 32 * s:32 * s + 32, :] = o
    return out
